# revision 72
# baseline (speedup 1.0000x reference)
"""GQA attention kernel for Trainium2, sharded over 8 NeuronCores.

Problem: B=2, S=2048, HIDDEN=2048, 16 Q heads / 4 KV heads, head_dim=128,
causal mask, f32.

Sharding: core = 4*b + g  (b in {0,1}: batch / data parallel;
g in {0..3}: KV-head group / tensor parallel). Each core computes its
4 Q heads + 1 KV head for one batch element and produces the partial
output projection (pre-bias). Host sums the 4 TP partials per batch and
adds wo_b.

Layout strategy (everything contracts over the partition dim, and all
PE streams are N=512 wide):
- x host-transposed to xT [H, S] and cast to bf16 (with wq/wk/wv/wo);
  projections run per 512-wide s-chunk as single-PSUM-bank passes
  (K, V, Q0..Q3). Chunk 0 is the ramp: weights issue on the ACT/gp DMA
  queues in parallel with the x stream on SP, K+V run block-major
  tracking x arrival, and all four Q heads backfill their first-half
  k-blocks into the second half of the stream. Later chunks' x
  descriptors are prefetched a full attention chunk ahead so the
  in-order PE never fronts a matmul whose data was just requested.
- Projections directly produce qT [d, s] and kT [d, s] (bf16);
  v is PE-transposed once per chunk into v [s, d] blocks (bf16).
- Scores are computed TRANSPOSED: sT[sk, sq] = kT_blk.T @ qT_chunk, so
  exp(sT) = pT lands in exactly the layout PV needs:
  houtT[d, sq] += v_blk.T @ pT_blk. No per-tile transposes at all.
- Causal: sk-blocks above the diagonal are skipped; diagonal blocks are
  sliced on the moving dim; their strictly-lower triangle is zeroed
  post-exp by an affine_select on the (otherwise idle) Pool engine.
- Softmax skips max-subtraction (scores bounded, exp exact-safe in f32).
  Row sums: pT tiles accumulate into colsum [128, sq] on DVE (bf16),
  then one ones-matrix matmul both reduces over partitions and
  broadcasts the row-sum to all 128 partitions; reciprocal_approx_fast
  on DVE (the exact InstReciprocal costs 3.4us/tile on HW); the
  1/rowsum multiply is fused into the houtT PSUM->SBUF move (DVE).
- Output projection: out[sq, H] = sum_h houtT_h[:, sq_blk].T @ wo_h;
  PSUM->SBUF casts to bf16 alternate between DVE and ACT so neither
  helper engine paces the wo pipeline; bf16 partials DMA'd out per
  128-row block, the last block in fine slices across two issue queues
  so the final drain is short. Host sums the 4 TP partials in f64+wo_b.
- PSUM banks (8 x 2KB): pj2 (proj passes; chunk-3 score lookahead
  borrows them), s2 (scores), ho1, m1 (vt+rowsum), wo2.
- Emission interleaves proj(c+1) passes and wo blocks between attn(c)
  heads so the PE always has independent matmuls in flight; wo blocks
  are deferred across chunk windows (WSCHED) so every attention window
  ends up PE-bound — attn(3) is exp/ACT-bound on its own, so it
  absorbs extra wo work for free while attn(2) sheds it 1:1.
  (NOTE: emission order IS program order — every read must be emitted
  after the write it depends on; the Tile scheduler only reorders
  independent ops).
Measured on HW (NTFF): ~241-242us vs 465us for the previous version
(ambient load on the shared device adds up to ~40us in bad windows).
"""

import os
import sys

import numpy as np
import ml_dtypes

for _p in ("/opt/trn_rl_repo", "/root/.axon_site/_ro/trn_rl_repo"):
    if os.path.isdir(_p) and _p not in sys.path:
        sys.path.append(_p)

import concourse.bacc as bacc
import concourse.bass as bass
import concourse.mybir as mybir
import concourse.tile as tile
from concourse.bass_utils import run_bass_kernel_spmd
from concourse.masks import make_identity

F32 = mybir.dt.float32
F32R = mybir.dt.float32r
BF16 = mybir.dt.bfloat16
AF = mybir.ActivationFunctionType
MUL = mybir.AluOpType.mult

B, S, H = 2, 2048, 2048
D = 128            # head dim
NHL = 4            # q heads per core
OL = NHL * D       # local q/o width = 512
P = 128            # partitions
NKB = H // P       # 16 contraction blocks for projections
NSB = S // P       # 16 sequence blocks of 128
CH = 512           # s-chunk width
NCH = S // CH      # 4 chunks
QSCALE = 1.0 / np.sqrt(D)

_NC = None


def _body(nc, tc, t):
    ctx_pools = []

    def pool(name, bufs, space=None):
        kw = dict(name=name, bufs=bufs)
        if space is not None:
            kw["space"] = space
        p = tc.tile_pool(**kw)
        ctx_pools.append(p)
        return p.__enter__()

    const = pool("const", 1)
    wpool = pool("wts", 1)
    gp = nc.gpsimd
    xpool = pool("xstream", 10)   # two full prefetched chunks + slack
    x0pool = pool("xstream0", 8)   # chunk 0: NKB//GRP0 tiles all stay live
    qkv = pool("qkv", 1)
    ppool = pool("pbuf", 24)     # pT tiles (chunk 3 peaks ~18 live)
    vpool = pool("vtmp", 2)      # vT_sb staging
    cpool = pool("csum", 3)      # colsum accumulators
    rpool = pool("recip", 2)     # broadcast reciprocal rows
    hpool = pool("houts", 14)    # normalized houtT: chunk-1 ho stays
                                 # live into attn(3) (wo(1) deferral)
    opool = pool("outbuf", 2)    # output staging
    ps_pj = pool("pspj", 2, bass.MemorySpace.PSUM)   # projection passes
    ps_s = pool("pss", 2, bass.MemorySpace.PSUM)     # score tiles
    ps_h = pool("psh", 1, bass.MemorySpace.PSUM)     # houtT accumulators
    ps_m = pool("psm", 1, bass.MemorySpace.PSUM)     # vt / rowsum
    ps_wo = pool("pswo", 2, bass.MemorySpace.PSUM)   # wo passes

    # ---- constants (DMAs deferred into proj_chunk(0) so they don't
    #      delay the critical wk/x descriptors at startup) ----
    ident = const.tile([P, P], BF16, tag="ident")
    make_identity(nc, ident[:])

    ones = const.tile([P, P], BF16, tag="ones")
    bq = const.tile([P, NHL], F32, tag="bq")
    bk = const.tile([P, 1], F32, tag="bk")
    bv = const.tile([P, 1], F32, tag="bv")

    # ---- weights ----
    wq = wpool.tile([P, NKB * OL], BF16, tag="wq")
    wk = wpool.tile([P, NKB * D], BF16, tag="wk")
    wv = wpool.tile([P, NKB * D], BF16, tag="wv")
    wo = wpool.tile([P, NHL * H], BF16, tag="wo")

    # ---- persistent activations ----
    qT = [qkv.tile([P, S], BF16, tag=f"qT{h}", name=f"qT{h}")
          for h in range(NHL)]
    kT = qkv.tile([P, S], BF16, tag="kT", name="kT")
    vblk = qkv.tile([P, S], BF16, tag="vblk", name="vblk")  # [sk, 16*d]

    # ============ phase 1: projections for s-chunk n ============
    GRP = 4            # k-blocks per x DMA descriptor (1 MiB each)
    GRP0 = 2           # chunk-0 descriptor width (k-blocks per descriptor)

    def x_stream(n):
        """Issue the x descriptors for chunk n; returns k -> slice view.

        Chunk 0 is the ramp: 2-k-block descriptors alternate between the
        SP and ACT HWDGE queues (double issue rate), with the wk quarter
        needed by each even pair issued just ahead of it on SP.
        """
        grp = GRP0 if n == 0 else GRP
        xgs = []
        wk_issued = 0
        for g in range(NKB // grp):
            if n == 0 and wk_issued < (g + 1) * grp:
                # wk quarters ride between x descriptors on SP so K
                # matmul k can fire as soon as its (wk, x) pair lands.
                hi = min(wk_issued + 4, NKB)
                nc.sync.dma_start(
                    out=wk[:, wk_issued * D:hi * D]
                    .rearrange("p (k d) -> p k d", d=D),
                    in_=t["wkT"][wk_issued * P:hi * P, :]
                    .rearrange("(k p) d -> p k d", p=P))
                wk_issued = hi
            xg = (x0pool.tile([P, grp * CH], BF16, tag="xg0", name="xg0")
                  if n == 0 else
                  xpool.tile([P, grp * CH], BF16, tag="xg", name="xg"))
            # chunk 0 alternates descriptors across the SP and ACT DMA
            # rings: the ramp's issue path is ring-credit limited (HW
            # shows DMA issues blocking 10-48us on credits), so two
            # rings drain the stream faster than one.
            eng = nc.scalar if (n == 0 and g % 2 == 1) else nc.sync
            eng.dma_start(
                out=xg[:].rearrange("p (k s) -> p k s", s=CH),
                in_=t["xT"][g * grp * P:(g + 1) * grp * P,
                            n * CH:(n + 1) * CH]
                .rearrange("(k p) s -> p k s", p=P))
            xgs.append(xg)

        def xt(k):
            return xgs[k // grp][:, (k % grp) * CH:(k % grp + 1) * CH]
        return xt

    def proj_chunk0():
        """Chunk 0: weights issue on ACT/DVE queues in parallel with the
        x stream on SP; K+V run block-major so the PE tracks x arrival."""
        # off-SP weight issue (ACT: wv + small consts; DVE: wq, wo)
        nc.scalar.dma_start(out=wv[:].rearrange("p (k d) -> p k d", d=D),
                            in_=t["wvT"][:].rearrange("(k p) d -> p k d",
                                                      p=P))
        for g in range(4):
            gp.dma_start(
                out=wq[:, g * 4 * OL:(g + 1) * 4 * OL]
                .rearrange("p (k d) -> p k d", d=OL),
                in_=t["wqT"][g * 4 * P:(g + 1) * 4 * P, :]
                .rearrange("(k p) d -> p k d", p=P))
        for cc in range(NHL):
            gp.dma_start(out=wo[:, cc * H:(cc + 1) * H],
                         in_=t["woT"][cc * P:(cc + 1) * P, :])
        xt = x_stream(0)
        # small consts trail the ramp-critical descriptors on ACT
        nc.scalar.dma_start(out=bk[:], in_=t["bk"][:])
        nc.scalar.dma_start(out=bv[:], in_=t["bv"][:])
        nc.scalar.dma_start(out=bq[:], in_=t["bq"][:].rearrange("a p -> p a"))
        nc.scalar.dma_start(out=ones[:], in_=t["ones"][:])
        # K+V block-major, with all four Q heads' first-half blocks
        # injected in the second half of the stream (by then wq has
        # landed on the gp queue), so the PE tracks x-stream arrival.
        # Chunk 0 briefly borrows every PSUM bank: K,V on pj; Q0,Q1 on
        # s; Q2 on ho; Q3 on wo (all idle until attention starts).
        k_ps = ps_pj.tile([P, CH], F32, tag="pj", name="kps")
        v_ps = ps_pj.tile([P, CH], F32, tag="pj", name="vps")
        q_ps = [ps_s.tile([P, CH], F32, tag="s", name="qps"),
                ps_s.tile([P, CH], F32, tag="s", name="qps"),
                ps_h.tile([P, CH], F32, tag="ho", name="qps"),
                ps_wo.tile([P, CH], F32, tag="wo", name="qps")]

        def qmm(h, k, start, stop):
            nc.tensor.matmul(
                q_ps[h][:], wq[:, k * OL + h * D: k * OL + (h + 1) * D],
                xt(k), start=start, stop=stop)

        # Q backfill batches are emitted BEFORE each k's K/V matmuls:
        # the PE is in-order, so only work emitted ahead of a stalling
        # K(k) can execute while its x descriptor is still in flight.
        # Batch 1 lags the stream by LAG blocks (wq's first quarter
        # must land first); batch 2 (lag 2) doubles the resident work
        # during the final pair-waits of the stream.
        LAG = 6
        for k in range(NKB):
            if k >= LAG:
                kk = k - LAG
                for h in range(NHL):
                    qmm(h, kk, start=(kk == 0), stop=False)
            if k >= NKB - 4:
                kk2 = k - 2           # covers blocks 10..13
                for h in range(NHL):
                    qmm(h, kk2, start=False, stop=False)
            nc.tensor.matmul(k_ps[:], wk[:, k * D:(k + 1) * D], xt(k),
                             start=(k == 0), stop=(k == NKB - 1))
            nc.tensor.matmul(v_ps[:], wv[:, k * D:(k + 1) * D], xt(k),
                             start=(k == 0), stop=(k == NKB - 1))
        nc.scalar.activation(kT[:, 0:CH], k_ps[:],
                             AF.Identity, bias=bk[:, 0:1], scale=1.0)
        vT_sb = vpool.tile([P, CH], BF16, tag="vT", name="vT_sb")
        nc.scalar.activation(vT_sb[:], v_ps[:], AF.Identity,
                             bias=bv[:, 0:1], scale=1.0)
        # finish Q0/Q1 on resident x, transpose v, then Q2/Q3
        for h in range(2):
            for k in range(NKB - 2, NKB):
                qmm(h, k, start=False, stop=(k == NKB - 1))
            nc.scalar.activation(qT[h][:, 0:CH], q_ps[h][:],
                                 AF.Identity, bias=bq[:, h:h + 1],
                                 scale=QSCALE)
        vt_ps = ps_m.tile([P, CH], BF16, tag="m", name="vtps")
        for jj in range(CH // P):
            nc.tensor.transpose(vt_ps[:, jj * P:(jj + 1) * P],
                                vT_sb[:, jj * P:(jj + 1) * P], ident[:])
        nc.vector.tensor_copy(vblk[:, 0:CH], vt_ps[:])
        for h in range(2, NHL):
            for k in range(NKB - 2, NKB):
                qmm(h, k, start=False, stop=(k == NKB - 1))
            nc.scalar.activation(qT[h][:, 0:CH], q_ps[h][:],
                                 AF.Identity, bias=bq[:, h:h + 1],
                                 scale=QSCALE)

    def proj_chunk(n, xt):
        # x descriptors were prefetched by the caller (x_stream(n) at
        # the top of the PREVIOUS attention chunk) so the in-order PE
        # never fronts a K-pass matmul whose data was just requested.
        # K pass
        k_ps = ps_pj.tile([P, CH], F32, tag="pj", name="kps")
        for k in range(NKB):
            nc.tensor.matmul(k_ps[:], wk[:, k * D:(k + 1) * D], xt(k),
                             start=(k == 0), stop=(k == NKB - 1))
        nc.scalar.activation(kT[:, n * CH:(n + 1) * CH], k_ps[:],
                             AF.Identity, bias=bk[:, 0:1], scale=1.0)
        yield
        # V pass
        v_ps = ps_pj.tile([P, CH], F32, tag="pj", name="vps")
        for k in range(NKB):
            nc.tensor.matmul(v_ps[:], wv[:, k * D:(k + 1) * D], xt(k),
                             start=(k == 0), stop=(k == NKB - 1))
        vT_sb = vpool.tile([P, CH], BF16, tag="vT", name="vT_sb")
        nc.scalar.activation(vT_sb[:], v_ps[:], AF.Identity,
                             bias=bv[:, 0:1], scale=1.0)
        vt_ps = ps_m.tile([P, CH], BF16, tag="m", name="vtps")
        for jj in range(CH // P):
            nc.tensor.transpose(vt_ps[:, jj * P:(jj + 1) * P],
                                vT_sb[:, jj * P:(jj + 1) * P], ident[:])
        nc.vector.tensor_copy(vblk[:, n * CH:(n + 1) * CH], vt_ps[:])
        yield
        # Q passes
        for h in range(NHL):
            q_ps = ps_pj.tile([P, CH], F32, tag="pj", name="qps")
            for k in range(NKB):
                nc.tensor.matmul(
                    q_ps[:], wq[:, k * OL + h * D: k * OL + (h + 1) * D],
                    xt(k), start=(k == 0), stop=(k == NKB - 1))
            nc.scalar.activation(qT[h][:, n * CH:(n + 1) * CH], q_ps[:],
                                 AF.Identity, bias=bq[:, h:h + 1],
                                 scale=QSCALE)
            yield

    # ============ phase 2: attention for s-chunk c ============
    # Per (head h, chunk c): nsk = 4c+4 key blocks. Block j < 4c is full
    # (sq cols 0:512); diagonal block j = 4c+e covers cols lo=e*128 .. 512.
    def score_exp(h, c, j):
        """sT -> exp -> pT for one (h, c, key-block j). Returns (pT, lo)."""
        e = j - 4 * c
        lo = max(e, 0) * P
        # chunk 3 has no interleaved projection, so its score pipeline
        # also rotates through the idle ps_pj banks for more lookahead
        sp = ps_pj if (c == NCH - 1 and j % 2 == 1) else ps_s
        s_ps = sp.tile([P, CH], F32, tag="pj" if sp is ps_pj else "s",
                       name="sps")
        nc.tensor.matmul(s_ps[:, lo:CH], kT[:, j * P:(j + 1) * P],
                         qT[h][:, c * CH + lo:(c + 1) * CH],
                         start=True, stop=True)
        p_sb = ppool.tile([P, CH], BF16, tag="p", name="p_sb")
        nc.scalar.activation(p_sb[:, lo:CH], s_ps[:, lo:CH], AF.Exp)
        if e >= 0:
            # zero the strictly-lower triangle (sq < sk) of the diagonal
            # 128x128 block on Pool: keep where (col - row) >= 0.
            gp.affine_select(
                out=p_sb[:, lo:lo + P], in_=p_sb[:, lo:lo + P],
                compare_op=mybir.AluOpType.is_ge, fill=0.0,
                base=0, pattern=[[1, P]], channel_multiplier=-1)
        return p_sb, lo

    def attn_chunk(c):
        nsk = 4 * c + 4
        pend = [score_exp(0, c, j) for j in range(nsk)]
        for h in range(NHL):
            nxt = []
            ho_ps = ps_h.tile([P, CH], F32, tag="ho", name="hops")
            colsum = cpool.tile([P, CH], BF16, tag="cs", name="colsum")
            for j in range(nsk):
                if h + 1 < NHL:
                    nxt.append(score_exp(h + 1, c, j))
                p_sb, lo = pend[j]
                # colsum accumulate (DVE)
                if j == 0:
                    nc.vector.tensor_copy(colsum[:], p_sb[:])
                else:
                    nc.vector.tensor_add(colsum[:, lo:CH], colsum[:, lo:CH],
                                         p_sb[:, lo:CH])
                # PV accumulate (PE)
                nc.tensor.matmul(ho_ps[:, lo:CH],
                                 vblk[:, j * P:(j + 1) * P],
                                 p_sb[:, lo:CH],
                                 start=(j == 0), stop=(j == nsk - 1))
            pend = nxt
            # rowsum reduce+broadcast (PE), reciprocal (DVE),
            # normalize fused into the PSUM->SBUF move (DVE).
            r_ps = ps_m.tile([P, CH], F32, tag="m", name="rps")
            nc.tensor.matmul(r_ps[:], ones[:], colsum[:],
                             start=True, stop=True)
            rb_sb = rpool.tile([P, CH], F32, tag="rb", name="rb_sb")
            # ~5x faster than reciprocal(); rowsums are positive and
            # well inside the safe range (no denorm/inf edge cases)
            nc.vector.reciprocal_approx_fast(rb_sb[:], r_ps[:])
            ho_sb = hpool.tile([P, CH], BF16, tag="hT", name="ho_sb")
            nc.vector.tensor_tensor(out=ho_sb[:], in0=ho_ps[:],
                                    in1=rb_sb[:], op=MUL)
            yield ho_sb

    # ============ phase 3: output projection for s-chunk c ============
    def wo_chunk(c, ho_sbs):
        for iq in range(4):          # 128-row sq blocks within the chunk
            # the very last block ships per 512-col slice so the final
            # DMA only drains 128KB after the last copy
            fine = (c == NCH - 1 and iq == 3)
            out_sb = opool.tile([P, H], BF16, tag="out", name="out_sb")
            for nn in range(H // CH):
                wo_ps = ps_wo.tile([P, CH], F32, tag="wo", name="wops")
                for hh in range(NHL):
                    nc.tensor.matmul(
                        wo_ps[:], ho_sbs[hh][:, iq * P:(iq + 1) * P],
                        wo[:, hh * H + nn * CH: hh * H + (nn + 1) * CH],
                        start=(hh == 0), stop=(hh == NHL - 1))
                # Spread the PSUM->SBUF casts across DVE and ACT so
                # neither helper engine becomes the wo-pipeline pacer —
                # EXCEPT blocks that execute inside attn(3)'s window
                # (wo(2) fully, wo(1) blocks 2-3 after the deferral
                # below): that window is ACT-bound (exp-dominated), so
                # their casts stay off ACT.
                on_act = (False if (c == 2 or (c == 1 and iq >= 2))
                          else (nn % 2 == 1))
                parts = ([(0, CH)] if not (fine and nn == H // CH - 1)
                         else [(0, CH // 2), (CH // 2, CH)])
                for pi, (a, b) in enumerate(parts):
                    if on_act:
                        nc.scalar.copy(out_sb[:, nn * CH + a:nn * CH + b],
                                       wo_ps[:, a:b])
                    else:
                        nc.vector.tensor_copy(
                            out_sb[:, nn * CH + a:nn * CH + b],
                            wo_ps[:, a:b])
                    if fine:
                        # rotate issue queues so the tail descriptor
                        # issues overlap instead of serializing on SP
                        deng = nc.scalar if (nn + pi) % 2 == 1 else nc.sync
                        deng.dma_start(
                            out=t["outp"][(c * 4 + iq) * P:
                                          (c * 4 + iq + 1) * P,
                                          nn * CH + a:nn * CH + b],
                            in_=out_sb[:, nn * CH + a:nn * CH + b])
            if not fine:
                nc.sync.dma_start(
                    out=t["outp"][(c * 4 + iq) * P:(c * 4 + iq + 1) * P, :],
                    in_=out_sb[:])
            yield True

    # ============ emission: proj(c+1) overlaps attn(c)'s tail ============
    # proj(c+1)'s passes are interleaved between attn(c)'s heads so PE
    # always has independent projection matmuls when the exp-paced
    # attention pipeline would otherwise stall it; wo(c) follows.
    proj_chunk0()
    xts = {1: x_stream(1)}
    # wo blocks are PE filler: attn(2) is PE-bound while attn(3) is
    # ACT-bound (exp-dominated) with PE slack, so two of wo(1)'s blocks
    # defer from attn(2)'s window into attn(3)'s, where the extra PE
    # work is absorbed for free. proj(3) spreads across attn(2)'s heads
    # to cover the boundaries wo(1) no longer fills there.
    PSCHED = {0: [6, 0, 0, 0], 1: [6, 0, 0, 0], 2: [2, 2, 1, 1]}
    WSCHED = {0: [0, 0, 0, 0], 1: [1, 1, 1, 1],
              2: [1, 1, 0, 0], 3: [2, 2, 1, 1]}
    pend_wo = []                # FIFO of live wo_chunk generators
    for c in range(NCH):
        ap = attn_chunk(c)
        if c + 2 < NCH:
            xts[c + 2] = x_stream(c + 2)   # prefetch next-next chunk
        pp = proj_chunk(c + 1, xts[c + 1]) if c + 1 < NCH else None
        ho_sbs = []
        for h in range(NHL):
            ho_sbs.append(next(ap))
            if pp is not None:
                for _ in range(PSCHED[c][h]):
                    next(pp, None)
            for _ in range(WSCHED[c][h]):
                while pend_wo:
                    if next(pend_wo[0], None) is None:
                        pend_wo.pop(0)
                        continue
                    break
        if pp is not None:
            for _ in pp:
                pass
        pend_wo.append(wo_chunk(c, ho_sbs))
    for g in pend_wo:
        for _ in g:
            pass


def _build():
    nc = bacc.Bacc("TRN2", target_bir_lowering=False, debug=False,
                   num_devices=8)
    t = {}
    t["xT"] = nc.dram_tensor("xT", [H, S], BF16, kind="ExternalInput")
    t["wqT"] = nc.dram_tensor("wqT", [H, OL], BF16, kind="ExternalInput")
    t["wkT"] = nc.dram_tensor("wkT", [H, D], BF16, kind="ExternalInput")
    t["wvT"] = nc.dram_tensor("wvT", [H, D], BF16, kind="ExternalInput")
    t["woT"] = nc.dram_tensor("woT", [OL, H], BF16, kind="ExternalInput")
    t["bq"] = nc.dram_tensor("bq", [NHL, D], F32, kind="ExternalInput")
    t["bk"] = nc.dram_tensor("bk", [D, 1], F32, kind="ExternalInput")
    t["bv"] = nc.dram_tensor("bv", [D, 1], F32, kind="ExternalInput")
    t["ones"] = nc.dram_tensor("ones", [P, P], BF16, kind="ExternalInput")
    t["outp"] = nc.dram_tensor("outp", [S, H], BF16, kind="ExternalOutput")

    with tile.TileContext(nc) as tc:
        _body(nc, tc, t)
    nc.compile()
    return nc, t


def _get_nc():
    global _NC
    if _NC is None:
        _NC = _build()
    return _NC


def make_in_maps(x, wq_w, wq_b, wk_w, wk_b, wv_w, wv_b, wo_w):
    x = np.asarray(x, np.float32)
    wqT = np.ascontiguousarray(np.asarray(wq_w, np.float32).T)   # [H, 2048]
    wkT = np.ascontiguousarray(np.asarray(wk_w, np.float32).T)   # [H, 512]
    wvT = np.ascontiguousarray(np.asarray(wv_w, np.float32).T)
    woT = np.ascontiguousarray(np.asarray(wo_w, np.float32).T)   # [2048, H]
    in_maps = []
    for core in range(8):
        b, g = divmod(core, 4)
        in_maps.append({
            "xT": np.ascontiguousarray(x[b].T).astype(ml_dtypes.bfloat16),
            "wqT": np.ascontiguousarray(
                wqT[:, g * OL:(g + 1) * OL]).astype(ml_dtypes.bfloat16),
            "wkT": np.ascontiguousarray(
                wkT[:, g * D:(g + 1) * D]).astype(ml_dtypes.bfloat16),
            "wvT": np.ascontiguousarray(
                wvT[:, g * D:(g + 1) * D]).astype(ml_dtypes.bfloat16),
            "woT": np.ascontiguousarray(
                woT[g * OL:(g + 1) * OL, :]).astype(ml_dtypes.bfloat16),
            "bq": (np.asarray(wq_b, np.float32)[g * OL:(g + 1) * OL]
                   * QSCALE).reshape(NHL, D),
            "bk": np.asarray(wk_b, np.float32)[g * D:(g + 1) * D]
                  .reshape(D, 1),
            "bv": np.asarray(wv_b, np.float32)[g * D:(g + 1) * D]
                  .reshape(D, 1),
            "ones": np.ones((P, P), ml_dtypes.bfloat16),
        })
    return in_maps


def kernel(x, attention_mask, wq_w, wq_b, wk_w, wk_b, wv_w, wv_b, wo_w,
           wo_b, _trace=False, _trace_kwargs=None):
    nc, t = _get_nc()
    in_maps = make_in_maps(x, wq_w, wq_b, wk_w, wk_b, wv_w, wv_b, wo_w)
    res = run_bass_kernel_spmd(nc, in_maps, core_ids=list(range(8)),
                               trace=_trace,
                               **(_trace_kwargs or {}))
    wo_b = np.asarray(wo_b, np.float32)
    outs = []
    for b in range(B):
        acc = np.zeros((S, H), np.float64)
        for g in range(4):
            acc += res.results[4 * b + g]["outp"].astype(np.float64)
        outs.append((acc + wo_b[None, :]).astype(np.float32))
    out = np.stack(outs, axis=0)
    if _trace:
        kernel._last_results = res
    return out



# revision 73
# speedup vs baseline: 1.1732x; 1.1732x over previous
"""GQA attention kernel for Trainium2, sharded over 8 NeuronCores.

Problem: B=2, S=2048, HIDDEN=2048, 16 Q heads / 4 KV heads, head_dim=128,
causal mask, f32.

Sharding: core = 4*b + g  (b in {0,1}: batch / data parallel;
g in {0..3}: KV-head group / tensor parallel). Each core computes its
4 Q heads + 1 KV head for one batch element and produces the partial
output projection (pre-bias). Host sums the 4 TP partials per batch and
adds wo_b.

Layout strategy (everything contracts over the partition dim, and all
PE streams are N=512 wide):
- x host-transposed to xT [H, S] and cast to bf16 (with wq/wk/wv/wo);
  projections run per 512-wide s-chunk as single-PSUM-bank passes
  (K, V, Q0..Q3). Chunk 0 is the ramp: weights issue on the ACT/gp DMA
  queues in parallel with the x stream on SP, K+V run block-major
  tracking x arrival, and all four Q heads backfill their first-half
  k-blocks into the second half of the stream. Later chunks' x
  descriptors are prefetched a full attention chunk ahead so the
  in-order PE never fronts a matmul whose data was just requested.
- Projections directly produce qT [d, s] and kT [d, s] (bf16);
  v is PE-transposed once per chunk into v [s, d] blocks (bf16).
- Scores are computed TRANSPOSED: sT[sk, sq] = kT_blk.T @ qT_chunk, so
  exp(sT) = pT lands in exactly the layout PV needs:
  houtT[d, sq] += v_blk.T @ pT_blk. No per-tile transposes at all.
- Causal: sk-blocks above the diagonal are skipped; diagonal blocks are
  sliced on the moving dim; their strictly-lower triangle is zeroed
  post-exp by an affine_select on the (otherwise idle) Pool engine.
- Softmax skips max-subtraction (scores bounded, exp exact-safe in f32).
  Row sums: pT tiles accumulate into colsum [128, sq] on DVE (bf16),
  then one ones-matrix matmul both reduces over partitions and
  broadcasts the row-sum to all 128 partitions; reciprocal_approx_fast
  on DVE (the exact InstReciprocal costs 3.4us/tile on HW); the
  1/rowsum multiply is fused into the houtT PSUM->SBUF move (DVE).
- Output projection: out[sq, H] = sum_h houtT_h[:, sq_blk].T @ wo_h;
  PSUM->SBUF casts to bf16 alternate between DVE and ACT so neither
  helper engine paces the wo pipeline; bf16 partials DMA'd out per
  128-row block, the last block in fine slices across two issue queues
  so the final drain is short. Host sums the 4 TP partials in f64+wo_b.
- PSUM banks (8 x 2KB): pj2 (proj passes; chunk-3 score lookahead
  borrows them), s2 (scores), ho1, m1 (vt+rowsum), wo2.
- Emission interleaves proj(c+1) passes and wo blocks between attn(c)
  heads so the PE always has independent matmuls in flight; wo blocks
  are deferred across chunk windows (WSCHED) so every attention window
  ends up PE-bound — attn(3) is exp/ACT-bound on its own, so it
  absorbs extra wo work for free while attn(2) sheds it 1:1.
  (NOTE: emission order IS program order — every read must be emitted
  after the write it depends on; the Tile scheduler only reorders
  independent ops).
Measured on HW (NTFF): ~241-242us vs 465us for the previous version
(ambient load on the shared device adds up to ~40us in bad windows).
"""

import os
import sys

import numpy as np
import ml_dtypes

for _p in ("/opt/trn_rl_repo", "/root/.axon_site/_ro/trn_rl_repo"):
    if os.path.isdir(_p) and _p not in sys.path:
        sys.path.append(_p)

import concourse.bacc as bacc
import concourse.bass as bass
import concourse.mybir as mybir
import concourse.tile as tile
from concourse.bass_utils import run_bass_kernel_spmd
from concourse.masks import make_identity

F32 = mybir.dt.float32
F32R = mybir.dt.float32r
BF16 = mybir.dt.bfloat16
AF = mybir.ActivationFunctionType
MUL = mybir.AluOpType.mult

B, S, H = 2, 2048, 2048
D = 128            # head dim
NHL = 4            # q heads per core
OL = NHL * D       # local q/o width = 512
P = 128            # partitions
NKB = H // P       # 16 contraction blocks for projections
NSB = S // P       # 16 sequence blocks of 128
CH = 512           # s-chunk width
NCH = S // CH      # 4 chunks
QSCALE = 1.0 / np.sqrt(D)

_NC = None


def _body(nc, tc, t):
    ctx_pools = []

    def pool(name, bufs, space=None):
        kw = dict(name=name, bufs=bufs)
        if space is not None:
            kw["space"] = space
        p = tc.tile_pool(**kw)
        ctx_pools.append(p)
        return p.__enter__()

    const = pool("const", 1)
    wpool = pool("wts", 1)
    gp = nc.gpsimd
    xpool = pool("xstream", 10)   # two full prefetched chunks + slack
    x0pool = pool("xstream0", 8)   # chunk 0: NKB//GRP0 tiles all stay live
    qkv = pool("qkv", 1)
    ppool = pool("pbuf", 24)     # pT tiles (chunk 3 peaks ~18 live)
    vpool = pool("vtmp", 2)      # vT_sb staging
    cpool = pool("csum", 3)      # colsum accumulators
    rpool = pool("recip", 2)     # broadcast reciprocal rows
    hpool = pool("houts", 14)    # normalized houtT: chunk-1 ho stays
                                 # live into attn(3) (wo(1) deferral)
    opool = pool("outbuf", 2)    # output staging
    ps_pj = pool("pspj", 2, bass.MemorySpace.PSUM)   # projection passes
    ps_s = pool("pss", 2, bass.MemorySpace.PSUM)     # score tiles
    ps_h = pool("psh", 1, bass.MemorySpace.PSUM)     # houtT accumulators
    ps_m = pool("psm", 1, bass.MemorySpace.PSUM)     # vt / rowsum
    ps_wo = pool("pswo", 2, bass.MemorySpace.PSUM)   # wo passes

    # ---- constants (DMAs deferred into proj_chunk(0) so they don't
    #      delay the critical wk/x descriptors at startup) ----
    ident = const.tile([P, P], BF16, tag="ident")
    make_identity(nc, ident[:])

    ones = const.tile([P, P], BF16, tag="ones")
    bq = const.tile([P, NHL], F32, tag="bq")
    bk = const.tile([P, 1], F32, tag="bk")
    bv = const.tile([P, 1], F32, tag="bv")

    # ---- weights ----
    wq = wpool.tile([P, NKB * OL], BF16, tag="wq")
    wk = wpool.tile([P, NKB * D], BF16, tag="wk")
    wv = wpool.tile([P, NKB * D], BF16, tag="wv")
    wo = wpool.tile([P, NHL * H], BF16, tag="wo")

    # ---- persistent activations ----
    qT = [qkv.tile([P, S], BF16, tag=f"qT{h}", name=f"qT{h}")
          for h in range(NHL)]
    kT = qkv.tile([P, S], BF16, tag="kT", name="kT")
    vblk = qkv.tile([P, S], BF16, tag="vblk", name="vblk")  # [sk, 16*d]

    # ============ phase 1: projections for s-chunk n ============
    GRP = 4            # k-blocks per x DMA descriptor (1 MiB each)
    GRP0 = 2           # chunk-0 descriptor width (k-blocks per descriptor)

    def x_stream(n):
        """Issue the x descriptors for chunk n; returns k -> slice view.

        Chunk 0 is the ramp: 2-k-block descriptors alternate between the
        SP and ACT HWDGE queues (double issue rate), with the wk quarter
        needed by each even pair issued just ahead of it on SP.
        """
        grp = GRP0 if n == 0 else GRP
        xgs = []
        wk_issued = 0
        for g in range(NKB // grp):
            if n == 0 and wk_issued < (g + 1) * grp:
                # wk quarters ride between x descriptors on SP so K
                # matmul k can fire as soon as its (wk, x) pair lands.
                hi = min(wk_issued + 4, NKB)
                nc.sync.dma_start(
                    out=wk[:, wk_issued * D:hi * D]
                    .rearrange("p (k d) -> p k d", d=D),
                    in_=t["wkT"][wk_issued * P:hi * P, :]
                    .rearrange("(k p) d -> p k d", p=P))
                wk_issued = hi
            xg = (x0pool.tile([P, grp * CH], BF16, tag="xg0", name="xg0")
                  if n == 0 else
                  xpool.tile([P, grp * CH], BF16, tag="xg", name="xg"))
            nc.sync.dma_start(
                out=xg[:].rearrange("p (k s) -> p k s", s=CH),
                in_=t["xT"][g * grp * P:(g + 1) * grp * P,
                            n * CH:(n + 1) * CH]
                .rearrange("(k p) s -> p k s", p=P))
            xgs.append(xg)

        def xt(k):
            return xgs[k // grp][:, (k % grp) * CH:(k % grp + 1) * CH]
        return xt

    def proj_chunk0():
        """Chunk 0: weights issue on ACT/DVE queues in parallel with the
        x stream on SP; K+V run block-major so the PE tracks x arrival."""
        # off-SP weight issue (ACT: wv + small consts; DVE: wq, wo)
        nc.scalar.dma_start(out=wv[:].rearrange("p (k d) -> p k d", d=D),
                            in_=t["wvT"][:].rearrange("(k p) d -> p k d",
                                                      p=P))
        for g in range(4):
            gp.dma_start(
                out=wq[:, g * 4 * OL:(g + 1) * 4 * OL]
                .rearrange("p (k d) -> p k d", d=OL),
                in_=t["wqT"][g * 4 * P:(g + 1) * 4 * P, :]
                .rearrange("(k p) d -> p k d", p=P))
        for cc in range(NHL):
            gp.dma_start(out=wo[:, cc * H:(cc + 1) * H],
                         in_=t["woT"][cc * P:(cc + 1) * P, :])
        xt = x_stream(0)
        # small consts trail the ramp-critical descriptors on ACT
        nc.scalar.dma_start(out=bk[:], in_=t["bk"][:])
        nc.scalar.dma_start(out=bv[:], in_=t["bv"][:])
        nc.scalar.dma_start(out=bq[:], in_=t["bq"][:].rearrange("a p -> p a"))
        nc.scalar.dma_start(out=ones[:], in_=t["ones"][:])
        # K+V block-major, with all four Q heads' first-half blocks
        # injected in the second half of the stream (by then wq has
        # landed on the gp queue), so the PE tracks x-stream arrival.
        # Chunk 0 briefly borrows every PSUM bank: K,V on pj; Q0,Q1 on
        # s; Q2 on ho; Q3 on wo (all idle until attention starts).
        k_ps = ps_pj.tile([P, CH], F32, tag="pj", name="kps")
        v_ps = ps_pj.tile([P, CH], F32, tag="pj", name="vps")
        q_ps = [ps_s.tile([P, CH], F32, tag="s", name="qps"),
                ps_s.tile([P, CH], F32, tag="s", name="qps"),
                ps_h.tile([P, CH], F32, tag="ho", name="qps"),
                ps_wo.tile([P, CH], F32, tag="wo", name="qps")]

        def qmm(h, k, start, stop):
            nc.tensor.matmul(
                q_ps[h][:], wq[:, k * OL + h * D: k * OL + (h + 1) * D],
                xt(k), start=start, stop=stop)

        # Q backfill batches are emitted BEFORE each k's K/V matmuls:
        # the PE is in-order, so only work emitted ahead of a stalling
        # K(k) can execute while its x descriptor is still in flight.
        # Batch 1 lags the stream by LAG blocks (wq's first quarter
        # must land first); batch 2 (lag 2) doubles the resident work
        # during the final pair-waits of the stream.
        LAG = 6
        for k in range(NKB):
            if k >= LAG:
                kk = k - LAG
                for h in range(NHL):
                    qmm(h, kk, start=(kk == 0), stop=False)
            if k >= NKB - 4:
                kk2 = k - 2           # covers blocks 10..13
                for h in range(NHL):
                    qmm(h, kk2, start=False, stop=False)
            nc.tensor.matmul(k_ps[:], wk[:, k * D:(k + 1) * D], xt(k),
                             start=(k == 0), stop=(k == NKB - 1))
            nc.tensor.matmul(v_ps[:], wv[:, k * D:(k + 1) * D], xt(k),
                             start=(k == 0), stop=(k == NKB - 1))
        nc.scalar.activation(kT[:, 0:CH], k_ps[:],
                             AF.Identity, bias=bk[:, 0:1], scale=1.0)
        vT_sb = vpool.tile([P, CH], BF16, tag="vT", name="vT_sb")
        nc.scalar.activation(vT_sb[:], v_ps[:], AF.Identity,
                             bias=bv[:, 0:1], scale=1.0)
        # finish Q0/Q1 on resident x, transpose v, then Q2/Q3
        for h in range(2):
            for k in range(NKB - 2, NKB):
                qmm(h, k, start=False, stop=(k == NKB - 1))
            nc.scalar.activation(qT[h][:, 0:CH], q_ps[h][:],
                                 AF.Identity, bias=bq[:, h:h + 1],
                                 scale=QSCALE)
        vt_ps = ps_m.tile([P, CH], BF16, tag="m", name="vtps")
        for jj in range(CH // P):
            nc.tensor.transpose(vt_ps[:, jj * P:(jj + 1) * P],
                                vT_sb[:, jj * P:(jj + 1) * P], ident[:])
        nc.vector.tensor_copy(vblk[:, 0:CH], vt_ps[:])
        for h in range(2, NHL):
            for k in range(NKB - 2, NKB):
                qmm(h, k, start=False, stop=(k == NKB - 1))
            nc.scalar.activation(qT[h][:, 0:CH], q_ps[h][:],
                                 AF.Identity, bias=bq[:, h:h + 1],
                                 scale=QSCALE)

    def proj_chunk(n, xt):
        # x descriptors were prefetched by the caller (x_stream(n) at
        # the top of the PREVIOUS attention chunk) so the in-order PE
        # never fronts a K-pass matmul whose data was just requested.
        # K pass
        k_ps = ps_pj.tile([P, CH], F32, tag="pj", name="kps")
        for k in range(NKB):
            nc.tensor.matmul(k_ps[:], wk[:, k * D:(k + 1) * D], xt(k),
                             start=(k == 0), stop=(k == NKB - 1))
        nc.scalar.activation(kT[:, n * CH:(n + 1) * CH], k_ps[:],
                             AF.Identity, bias=bk[:, 0:1], scale=1.0)
        yield
        # V pass
        v_ps = ps_pj.tile([P, CH], F32, tag="pj", name="vps")
        for k in range(NKB):
            nc.tensor.matmul(v_ps[:], wv[:, k * D:(k + 1) * D], xt(k),
                             start=(k == 0), stop=(k == NKB - 1))
        vT_sb = vpool.tile([P, CH], BF16, tag="vT", name="vT_sb")
        nc.scalar.activation(vT_sb[:], v_ps[:], AF.Identity,
                             bias=bv[:, 0:1], scale=1.0)
        vt_ps = ps_m.tile([P, CH], BF16, tag="m", name="vtps")
        for jj in range(CH // P):
            nc.tensor.transpose(vt_ps[:, jj * P:(jj + 1) * P],
                                vT_sb[:, jj * P:(jj + 1) * P], ident[:])
        nc.vector.tensor_copy(vblk[:, n * CH:(n + 1) * CH], vt_ps[:])
        yield
        # Q passes
        for h in range(NHL):
            q_ps = ps_pj.tile([P, CH], F32, tag="pj", name="qps")
            for k in range(NKB):
                nc.tensor.matmul(
                    q_ps[:], wq[:, k * OL + h * D: k * OL + (h + 1) * D],
                    xt(k), start=(k == 0), stop=(k == NKB - 1))
            nc.scalar.activation(qT[h][:, n * CH:(n + 1) * CH], q_ps[:],
                                 AF.Identity, bias=bq[:, h:h + 1],
                                 scale=QSCALE)
            yield

    # ============ phase 2: attention for s-chunk c ============
    # Per (head h, chunk c): nsk = 4c+4 key blocks. Block j < 4c is full
    # (sq cols 0:512); diagonal block j = 4c+e covers cols lo=e*128 .. 512.
    def score_exp(h, c, j):
        """sT -> exp -> pT for one (h, c, key-block j). Returns (pT, lo)."""
        e = j - 4 * c
        lo = max(e, 0) * P
        # chunk 3 has no interleaved projection, so its score pipeline
        # also rotates through the idle ps_pj banks for more lookahead
        sp = ps_pj if (c == NCH - 1 and j % 2 == 1) else ps_s
        s_ps = sp.tile([P, CH], F32, tag="pj" if sp is ps_pj else "s",
                       name="sps")
        nc.tensor.matmul(s_ps[:, lo:CH], kT[:, j * P:(j + 1) * P],
                         qT[h][:, c * CH + lo:(c + 1) * CH],
                         start=True, stop=True)
        p_sb = ppool.tile([P, CH], BF16, tag="p", name="p_sb")
        nc.scalar.activation(p_sb[:, lo:CH], s_ps[:, lo:CH], AF.Exp)
        if e >= 0:
            # zero the strictly-lower triangle (sq < sk) of the diagonal
            # 128x128 block on Pool: keep where (col - row) >= 0.
            gp.affine_select(
                out=p_sb[:, lo:lo + P], in_=p_sb[:, lo:lo + P],
                compare_op=mybir.AluOpType.is_ge, fill=0.0,
                base=0, pattern=[[1, P]], channel_multiplier=-1)
        return p_sb, lo

    def attn_chunk(c):
        nsk = 4 * c + 4
        pend = [score_exp(0, c, j) for j in range(nsk)]
        for h in range(NHL):
            nxt = []
            ho_ps = ps_h.tile([P, CH], F32, tag="ho", name="hops")
            colsum = cpool.tile([P, CH], BF16, tag="cs", name="colsum")
            for j in range(nsk):
                if h + 1 < NHL:
                    nxt.append(score_exp(h + 1, c, j))
                p_sb, lo = pend[j]
                # colsum accumulate (DVE)
                if j == 0:
                    nc.vector.tensor_copy(colsum[:], p_sb[:])
                else:
                    nc.vector.tensor_add(colsum[:, lo:CH], colsum[:, lo:CH],
                                         p_sb[:, lo:CH])
                # PV accumulate (PE)
                nc.tensor.matmul(ho_ps[:, lo:CH],
                                 vblk[:, j * P:(j + 1) * P],
                                 p_sb[:, lo:CH],
                                 start=(j == 0), stop=(j == nsk - 1))
            pend = nxt
            # rowsum reduce+broadcast (PE), reciprocal (DVE),
            # normalize fused into the PSUM->SBUF move (DVE).
            r_ps = ps_m.tile([P, CH], F32, tag="m", name="rps")
            nc.tensor.matmul(r_ps[:], ones[:], colsum[:],
                             start=True, stop=True)
            rb_sb = rpool.tile([P, CH], F32, tag="rb", name="rb_sb")
            # ~5x faster than reciprocal(); rowsums are positive and
            # well inside the safe range (no denorm/inf edge cases)
            nc.vector.reciprocal_approx_fast(rb_sb[:], r_ps[:])
            ho_sb = hpool.tile([P, CH], BF16, tag="hT", name="ho_sb")
            nc.vector.tensor_tensor(out=ho_sb[:], in0=ho_ps[:],
                                    in1=rb_sb[:], op=MUL)
            yield ho_sb

    # ============ phase 3: output projection for s-chunk c ============
    def wo_chunk(c, ho_sbs):
        for iq in range(4):          # 128-row sq blocks within the chunk
            # the very last block ships per 512-col slice so the final
            # DMA only drains 128KB after the last copy
            fine = (c == NCH - 1 and iq == 3)
            out_sb = opool.tile([P, H], BF16, tag="out", name="out_sb")
            for nn in range(H // CH):
                wo_ps = ps_wo.tile([P, CH], F32, tag="wo", name="wops")
                for hh in range(NHL):
                    nc.tensor.matmul(
                        wo_ps[:], ho_sbs[hh][:, iq * P:(iq + 1) * P],
                        wo[:, hh * H + nn * CH: hh * H + (nn + 1) * CH],
                        start=(hh == 0), stop=(hh == NHL - 1))
                # Spread the PSUM->SBUF casts across DVE and ACT so
                # neither helper engine becomes the wo-pipeline pacer —
                # EXCEPT blocks that execute inside attn(3)'s window
                # (wo(2) fully, wo(1) blocks 2-3 after the deferral
                # below): that window is ACT-bound (exp-dominated), so
                # their casts stay off ACT.
                on_act = (False if (c == 2 or (c == 1 and iq >= 2))
                          else (nn % 2 == 1))
                parts = ([(0, CH)] if not (fine and nn == H // CH - 1)
                         else [(0, CH // 2), (CH // 2, CH)])
                for pi, (a, b) in enumerate(parts):
                    if on_act:
                        nc.scalar.copy(out_sb[:, nn * CH + a:nn * CH + b],
                                       wo_ps[:, a:b])
                    else:
                        nc.vector.tensor_copy(
                            out_sb[:, nn * CH + a:nn * CH + b],
                            wo_ps[:, a:b])
                    if fine:
                        # rotate issue queues so the tail descriptor
                        # issues overlap instead of serializing on SP
                        deng = nc.scalar if (nn + pi) % 2 == 1 else nc.sync
                        deng.dma_start(
                            out=t["outp"][(c * 4 + iq) * P:
                                          (c * 4 + iq + 1) * P,
                                          nn * CH + a:nn * CH + b],
                            in_=out_sb[:, nn * CH + a:nn * CH + b])
            if not fine:
                nc.sync.dma_start(
                    out=t["outp"][(c * 4 + iq) * P:(c * 4 + iq + 1) * P, :],
                    in_=out_sb[:])
            yield True

    # ============ emission: proj(c+1) overlaps attn(c)'s tail ============
    # proj(c+1)'s passes are interleaved between attn(c)'s heads so PE
    # always has independent projection matmuls when the exp-paced
    # attention pipeline would otherwise stall it; wo(c) follows.
    proj_chunk0()
    xts = {1: x_stream(1)}
    # wo blocks are PE filler: attn(2) is PE-bound while attn(3) is
    # ACT-bound (exp-dominated) with PE slack, so two of wo(1)'s blocks
    # defer from attn(2)'s window into attn(3)'s, where the extra PE
    # work is absorbed for free. proj(3) spreads across attn(2)'s heads
    # to cover the boundaries wo(1) no longer fills there.
    PSCHED = {0: [6, 0, 0, 0], 1: [6, 0, 0, 0], 2: [2, 2, 1, 1]}
    WSCHED = {0: [0, 0, 0, 0], 1: [1, 1, 1, 1],
              2: [1, 1, 0, 0], 3: [2, 2, 1, 1]}
    pend_wo = []                # FIFO of live wo_chunk generators
    for c in range(NCH):
        ap = attn_chunk(c)
        if c + 2 < NCH:
            xts[c + 2] = x_stream(c + 2)   # prefetch next-next chunk
        pp = proj_chunk(c + 1, xts[c + 1]) if c + 1 < NCH else None
        ho_sbs = []
        for h in range(NHL):
            ho_sbs.append(next(ap))
            if pp is not None:
                for _ in range(PSCHED[c][h]):
                    next(pp, None)
            for _ in range(WSCHED[c][h]):
                while pend_wo:
                    if next(pend_wo[0], None) is None:
                        pend_wo.pop(0)
                        continue
                    break
        if pp is not None:
            for _ in pp:
                pass
        pend_wo.append(wo_chunk(c, ho_sbs))
    for g in pend_wo:
        for _ in g:
            pass


def _build():
    nc = bacc.Bacc("TRN2", target_bir_lowering=False, debug=False,
                   num_devices=8)
    t = {}
    t["xT"] = nc.dram_tensor("xT", [H, S], BF16, kind="ExternalInput")
    t["wqT"] = nc.dram_tensor("wqT", [H, OL], BF16, kind="ExternalInput")
    t["wkT"] = nc.dram_tensor("wkT", [H, D], BF16, kind="ExternalInput")
    t["wvT"] = nc.dram_tensor("wvT", [H, D], BF16, kind="ExternalInput")
    t["woT"] = nc.dram_tensor("woT", [OL, H], BF16, kind="ExternalInput")
    t["bq"] = nc.dram_tensor("bq", [NHL, D], F32, kind="ExternalInput")
    t["bk"] = nc.dram_tensor("bk", [D, 1], F32, kind="ExternalInput")
    t["bv"] = nc.dram_tensor("bv", [D, 1], F32, kind="ExternalInput")
    t["ones"] = nc.dram_tensor("ones", [P, P], BF16, kind="ExternalInput")
    t["outp"] = nc.dram_tensor("outp", [S, H], BF16, kind="ExternalOutput")

    with tile.TileContext(nc) as tc:
        _body(nc, tc, t)
    nc.compile()
    return nc, t


def _get_nc():
    global _NC
    if _NC is None:
        _NC = _build()
    return _NC


def make_in_maps(x, wq_w, wq_b, wk_w, wk_b, wv_w, wv_b, wo_w):
    x = np.asarray(x, np.float32)
    wqT = np.ascontiguousarray(np.asarray(wq_w, np.float32).T)   # [H, 2048]
    wkT = np.ascontiguousarray(np.asarray(wk_w, np.float32).T)   # [H, 512]
    wvT = np.ascontiguousarray(np.asarray(wv_w, np.float32).T)
    woT = np.ascontiguousarray(np.asarray(wo_w, np.float32).T)   # [2048, H]
    in_maps = []
    for core in range(8):
        b, g = divmod(core, 4)
        in_maps.append({
            "xT": np.ascontiguousarray(x[b].T).astype(ml_dtypes.bfloat16),
            "wqT": np.ascontiguousarray(
                wqT[:, g * OL:(g + 1) * OL]).astype(ml_dtypes.bfloat16),
            "wkT": np.ascontiguousarray(
                wkT[:, g * D:(g + 1) * D]).astype(ml_dtypes.bfloat16),
            "wvT": np.ascontiguousarray(
                wvT[:, g * D:(g + 1) * D]).astype(ml_dtypes.bfloat16),
            "woT": np.ascontiguousarray(
                woT[g * OL:(g + 1) * OL, :]).astype(ml_dtypes.bfloat16),
            "bq": (np.asarray(wq_b, np.float32)[g * OL:(g + 1) * OL]
                   * QSCALE).reshape(NHL, D),
            "bk": np.asarray(wk_b, np.float32)[g * D:(g + 1) * D]
                  .reshape(D, 1),
            "bv": np.asarray(wv_b, np.float32)[g * D:(g + 1) * D]
                  .reshape(D, 1),
            "ones": np.ones((P, P), ml_dtypes.bfloat16),
        })
    return in_maps


def kernel(x, attention_mask, wq_w, wq_b, wk_w, wk_b, wv_w, wv_b, wo_w,
           wo_b, _trace=False, _trace_kwargs=None):
    nc, t = _get_nc()
    in_maps = make_in_maps(x, wq_w, wq_b, wk_w, wk_b, wv_w, wv_b, wo_w)
    res = run_bass_kernel_spmd(nc, in_maps, core_ids=list(range(8)),
                               trace=_trace,
                               **(_trace_kwargs or {}))
    wo_b = np.asarray(wo_b, np.float32)
    outs = []
    for b in range(B):
        acc = np.zeros((S, H), np.float64)
        for g in range(4):
            acc += res.results[4 * b + g]["outp"].astype(np.float64)
        outs.append((acc + wo_b[None, :]).astype(np.float32))
    out = np.stack(outs, axis=0)
    if _trace:
        kernel._last_results = res
    return out

